# revision 4
# baseline (speedup 1.0000x reference)
"""Trainium2 Bass kernel for nn_NlsqCond (ConvFlow NLSQ coupling layer).

Strategy: pure data parallel over batch B=256 -> 32 samples per core on 8
NeuronCores. Convs are computed as 3 shifted matmuls accumulating in PSUM
over a gap-column activation layout (34 columns per sample, zero guard
columns), so the k=3/pad=1 conv needs no boundary special-casing. Weights
are transposed host-side into lhsT layout and cast to bf16 (fp32 PSUM
accumulation); measured end-to-end error vs fp32 reference is ~1e-4.
The final projection is computed transposed ([cols, 10]) so the NLSQ
elementwise tail runs with full 128-partition parallelism; the per-sample
logdet partition-reduction is done with a small mask matmul.
"""

import math

import numpy as np

B, T, D, H, COND = 256, 64, 2, 512, 8
T2 = T // 2                      # 32
NCORES = 8
NB = B // NCORES                 # 32 samples per core
ST = T2 + 2                      # 34: per-sample column stride (zero gaps)
WCOLS = NB * ST                  # 1088
NG = 2                           # PSUM column groups per matmul set
GS = NB // NG                    # 16 samples per group
NCHUNK = NB * T2 // 128          # 8 column chunks of 128 for final proj
LOG_A = math.log(8.0 * math.sqrt(3.0) / 9.0 - 0.05)

_CACHE = {}


def _build_program():
    import concourse.bacc as bacc
    import concourse.mybir as mybir
    import concourse.tile as tile

    f32 = mybir.dt.float32
    bf16 = mybir.dt.bfloat16
    AF = mybir.ActivationFunctionType
    Alu = mybir.AluOpType

    nc = bacc.Bacc("TRN2", target_bir_lowering=False, debug=False)

    # ---- DRAM I/O ----
    d_z1T = nc.dram_tensor("z1T", [2, NB * T2], bf16, kind="ExternalInput")
    d_z2r = nc.dram_tensor("z2r", [128, NCHUNK, 2], f32, kind="ExternalInput")
    d_condT = nc.dram_tensor("condT", [2 * COND, NB], bf16, kind="ExternalInput")
    d_we = nc.dram_tensor("we", [2, H], bf16, kind="ExternalInput")
    d_wc1 = nc.dram_tensor("wc1", [2 * COND, H], bf16, kind="ExternalInput")
    d_wc2 = nc.dram_tensor("wc2", [4, 128, H], bf16, kind="ExternalInput")
    d_wc0 = nc.dram_tensor("wc0", [8, 128, 3, H], bf16, kind="ExternalInput")
    d_wcv = nc.dram_tensor("wcv", [4, 4, 128, 3, H], bf16, kind="ExternalInput")
    d_wout = nc.dram_tensor("wout", [4, 128, 10], bf16, kind="ExternalInput")
    d_bias = nc.dram_tensor("biases", [128, 8, 4], f32, kind="ExternalInput")
    d_brep = nc.dram_tensor("brep", [128, NCHUNK, 10], f32, kind="ExternalInput")
    d_mask = nc.dram_tensor("mask", [128, 4], f32, kind="ExternalInput")
    d_z2n = nc.dram_tensor("z2n", [128, NCHUNK, 2], f32, kind="ExternalOutput")
    d_ld = nc.dram_tensor("ld", [NCHUNK, 4], f32, kind="ExternalOutput")

    with tile.TileContext(nc) as tc:
        with (
            tc.tile_pool(name="w", bufs=1) as wp,
            tc.tile_pool(name="act", bufs=1) as ap_,
            tc.tile_pool(name="ew", bufs=1) as ewp,
            tc.tile_pool(name="ps", bufs=8, space="PSUM") as pp,
        ):
            def mk(pool, shape, dtype, tag):
                return pool.tile(shape, dtype, tag=tag, name=tag)

            # ---- SBUF loads ----
            wc0_sb = [mk(wp, [128, 3, H], bf16, f"wc0_{i}") for i in range(8)]
            for i in range(8):
                nc.sync.dma_start(wc0_sb[i][:], d_wc0.ap()[i])
            wcv_sb = [
                [mk(wp, [128, 3, H], bf16, f"wcv_{l}_{i}") for i in range(4)]
                for l in range(4)
            ]
            for l in range(4):
                for i in range(4):
                    nc.sync.dma_start(wcv_sb[l][i][:], d_wcv.ap()[l, i])
            we_sb = mk(wp, [2, H], bf16, "we")
            nc.sync.dma_start(we_sb[:], d_we.ap())
            wc1_sb = mk(wp, [2 * COND, H], bf16, "wc1")
            nc.sync.dma_start(wc1_sb[:], d_wc1.ap())
            wc2_sb = [mk(wp, [128, H], bf16, f"wc2_{i}") for i in range(4)]
            for i in range(4):
                nc.sync.dma_start(wc2_sb[i][:], d_wc2.ap()[i])
            wout_sb = [mk(wp, [128, 10], bf16, f"wout_{i}") for i in range(4)]
            for i in range(4):
                nc.sync.dma_start(wout_sb[i][:], d_wout.ap()[i])
            bias_sb = mk(wp, [128, 8, 4], f32, "bias")
            nc.sync.dma_start(bias_sb[:], d_bias.ap())
            brep_sb = mk(wp, [128, NCHUNK, 10], f32, "brep")
            nc.sync.dma_start(brep_sb[:], d_brep.ap())
            mask_sb = mk(wp, [128, 4], f32, "mask")
            nc.sync.dma_start(mask_sb[:], d_mask.ap())
            z1T_sb = mk(ap_, [2, NB * T2], bf16, "z1T")
            nc.sync.dma_start(z1T_sb[:], d_z1T.ap())
            condT_sb = mk(ap_, [2 * COND, NB], bf16, "condT")
            nc.sync.dma_start(condT_sb[:], d_condT.ap())
            z2r_sb = mk(ewp, [128, NCHUNK, 2], f32, "z2r")
            nc.sync.dma_start(z2r_sb[:], d_z2r.ap())

            def bias_ap(idx, mt):
                return bias_sb[:, idx, mt : mt + 1]

            # ---- activation ping-pong buffers (gap layout, pre-zeroed) ----
            actA = [mk(ap_, [128, WCOLS], bf16, f"A{i}") for i in range(8)]
            actB = [mk(ap_, [128, WCOLS], bf16, f"B{i}") for i in range(4)]
            actC = [mk(ap_, [128, WCOLS], bf16, f"C{i}") for i in range(4)]
            for t_ in actA + actB + actC:
                nc.gpsimd.memset(t_[:], 0.0)

            def valid(tl, g=None):
                v = tl[:].rearrange("p (s c) -> p s c", c=ST)
                if g is None:
                    return v[:, :, 1 : 1 + T2]
                return v[:, g * GS : (g + 1) * GS, 1 : 1 + T2]

            # ---- embedding: h = z1 @ we.T + b_embed -> actA[0..3] ----
            for g in range(NG):
                for mt in range(4):
                    ps = mk(pp, [128, 512], f32, "ps")
                    nc.tensor.matmul(
                        ps[:],
                        we_sb[:, mt * 128 : (mt + 1) * 128],
                        z1T_sb[:, g * 512 : (g + 1) * 512],
                        start=True,
                        stop=True,
                    )
                    nc.scalar.activation(
                        valid(actA[mt], g),
                        ps[:].rearrange("p (s t) -> p s t", t=T2),
                        AF.Identity,
                        bias=bias_ap(0, mt),
                    )

            # ---- cond MLP: c2 = relu(W2 relu(W1 c + b1) + b2) ----
            c1_sb = [mk(ap_, [128, NB], bf16, f"c1_{i}") for i in range(4)]
            c2_sb = [mk(ap_, [128, NB], bf16, f"c2_{i}") for i in range(4)]
            for mt in range(4):
                ps = mk(pp, [128, 512], f32, "ps")
                nc.tensor.matmul(
                    ps[:, :NB],
                    wc1_sb[:, mt * 128 : (mt + 1) * 128],
                    condT_sb[:],
                    start=True,
                    stop=True,
                )
                nc.scalar.activation(
                    c1_sb[mt][:], ps[:, :NB], AF.Relu, bias=bias_ap(1, mt)
                )
            for mt in range(4):
                ps = mk(pp, [128, 512], f32, "ps")
                for kt in range(4):
                    nc.tensor.matmul(
                        ps[:, :NB],
                        wc2_sb[kt][:, mt * 128 : (mt + 1) * 128],
                        c1_sb[kt][:],
                        start=(kt == 0),
                        stop=(kt == 3),
                    )
                nc.scalar.activation(
                    c2_sb[mt][:], ps[:, :NB], AF.Relu, bias=bias_ap(2, mt)
                )
            # broadcast c2 over positions t -> actA[4..7]
            for mt in range(4):
                src = c2_sb[mt][:].unsqueeze(2).broadcast_to([128, NB, T2])
                nc.scalar.copy(valid(actA[4 + mt]), src)

            # ---- conv stack ----
            # Matmul moving operands must be single-free-dim, so each conv
            # matmul streams a contiguous window of the gap layout; outputs
            # at gap positions are garbage and simply never read back.
            WINDOWS = [(0, 15), (15, 15), (30, 2)]  # (sample base, n samples)
            srcs, ktn, wts = actA, 8, wc0_sb
            for L in range(5):
                dsts = actB if L % 2 == 0 else actC
                for mt in range(4):
                    pss = [mk(pp, [128, 512], f32, "ps") for _ in WINDOWS]
                    nacc = 3 * ktn
                    i = 0
                    for k in range(3):
                        for kt in range(ktn):
                            lhsT = wts[kt][:, k, mt * 128 : (mt + 1) * 128]
                            for wi, (sb, ns) in enumerate(WINDOWS):
                                n = ns * ST - 2
                                base = sb * ST + k
                                nc.tensor.matmul(
                                    pss[wi][:, :n],
                                    lhsT,
                                    srcs[kt][:, base : base + n],
                                    start=(i == 0),
                                    stop=(i == nacc - 1),
                                )
                            i += 1
                    for wi, (sb, ns) in enumerate(WINDOWS):
                        pv_ = pss[wi][:, : ns * ST].rearrange(
                            "p (s c) -> p s c", c=ST
                        )[:, :, 0:T2]
                        dv_ = dsts[mt][:].rearrange("p (s c) -> p s c", c=ST)[
                            :, sb : sb + ns, 1 : 1 + T2
                        ]
                        nc.scalar.activation(
                            dv_, pv_, AF.Relu, bias=bias_ap(3 + L, mt)
                        )
                srcs, ktn = dsts, 4
                if L < 4:
                    wts = wcv_sb[L]

            # ---- compact the final activation (drop gap columns) ----
            h5 = [mk(ap_, [128, NB * T2], bf16, f"h5_{i}") for i in range(4)]
            for kt in range(4):
                nc.scalar.copy(
                    h5[kt][:].rearrange("p (s t) -> p s t", t=T2),
                    valid(srcs[kt]),
                )

            # ---- final projection (transposed): out[col, 10] ----
            pso = mk(pp, [128, 512], f32, "ps")
            for j in range(NCHUNK):
                for kt in range(4):
                    nc.tensor.matmul(
                        pso[:, j * 10 : (j + 1) * 10],
                        h5[kt][:, j * 128 : (j + 1) * 128],
                        wout_sb[kt][:],
                        start=(kt == 0),
                        stop=(kt == 3),
                    )

            # ---- NLSQ elementwise tail ----
            def ew(tag):
                return mk(ewp, [128, NCHUNK, 2], f32, tag)

            params = mk(ewp, [128, NCHUNK, 10], f32, "params")
            nc.vector.tensor_add(
                params[:],
                pso[:, : NCHUNK * 10].rearrange("p (j q) -> p j q", q=10),
                brep_sb[:],
            )
            pv = params[:].rearrange("p j (a q) -> p j a q", q=5)
            P0, P1, P2, P3, P4 = (pv[:, :, :, i] for i in range(5))

            loga_sb = mk(ewp, [128, 1], f32, "loga")
            nc.vector.memset(loga_sb[:], LOG_A)

            b_ = ew("b_")
            nc.scalar.activation(b_[:], P1, AF.Exp, scale=0.4)
            d_ = ew("d_")
            nc.scalar.activation(d_[:], P3, AF.Exp, scale=0.4)
            th = ew("th")
            nc.scalar.activation(th[:], P2, AF.Tanh, scale=0.3)
            u = ew("u")
            nc.vector.tensor_tensor(u[:], P1, P3, op=Alu.subtract)
            e = ew("e")
            nc.scalar.activation(e[:], u[:], AF.Exp, scale=0.4, bias=loga_sb[:])
            c_ = ew("c_")
            nc.vector.tensor_mul(c_[:], th[:], e[:])
            t1 = ew("t1")
            nc.vector.tensor_mul(t1[:], d_[:], z2r_sb[:])
            arg = ew("arg")
            nc.vector.tensor_add(arg[:], t1[:], P4)
            sq = ew("sq")
            nc.vector.tensor_mul(sq[:], arg[:], arg[:])
            den = ew("den")
            nc.vector.tensor_scalar_add(den[:], sq[:], 1.0)
            rcp = ew("rcp")
            nc.vector.reciprocal(rcp[:], den[:])
            t2 = ew("t2")
            nc.vector.tensor_mul(t2[:], b_[:], z2r_sb[:])
            t3 = ew("t3")
            nc.vector.tensor_mul(t3[:], c_[:], rcp[:])
            s1 = ew("s1")
            nc.vector.tensor_add(s1[:], P0, t2[:])
            z2n_sb = ew("z2n_sb")
            nc.vector.tensor_add(z2n_sb[:], s1[:], t3[:])
            nc.sync.dma_start(d_z2n.ap(), z2n_sb[:])

            t4 = ew("t4")
            nc.vector.tensor_mul(t4[:], c_[:], d_[:])
            t5 = ew("t5")
            nc.vector.tensor_mul(t5[:], t4[:], arg[:])
            t6 = ew("t6")
            nc.vector.tensor_mul(t6[:], t5[:], rcp[:])
            t7 = ew("t7")
            nc.vector.tensor_mul(t7[:], t6[:], rcp[:])
            inner = ew("inner")
            nc.vector.scalar_tensor_tensor(
                inner[:], t7[:], -2.0, b_[:], op0=Alu.mult, op1=Alu.add
            )
            lg = ew("lg")
            nc.scalar.activation(lg[:], inner[:], AF.Ln)

            lg2 = mk(ewp, [128, NCHUNK], f32, "lg2")
            nc.vector.tensor_add(lg2[:], lg[:, :, 0], lg[:, :, 1])
            psl = mk(pp, [128, 512], f32, "ps")
            nc.tensor.matmul(
                psl[:NCHUNK, :4], lg2[:], mask_sb[:], start=True, stop=True
            )
            ld_sb = mk(ewp, [NCHUNK, 4], f32, "ld_sb")
            nc.vector.tensor_copy(ld_sb[:], psl[:NCHUNK, :4])
            nc.sync.dma_start(d_ld.ap(), ld_sb[:])

    nc.compile()
    return nc


def _get_program():
    if "nc" not in _CACHE:
        _CACHE["nc"] = _build_program()
    return _CACHE["nc"]


def _host_inputs(inputs):
    import ml_dtypes

    bf16 = ml_dtypes.bfloat16
    f32 = np.float32

    x = np.asarray(inputs["x"], f32)
    cond = np.asarray(inputs["cond"], f32)

    weT = np.ascontiguousarray(inputs["w_embed"][:, :2].T).astype(bf16)
    wc1T = np.ascontiguousarray(inputs["w_c1"].T).astype(bf16)
    wc2T = np.ascontiguousarray(inputs["w_c2"].T).reshape(4, 128, H).astype(bf16)
    wc0 = (
        np.ascontiguousarray(np.transpose(inputs["w_conv0"], (1, 2, 0)))
        .reshape(8, 128, 3, H)
        .astype(bf16)
    )
    wcv = np.stack(
        [
            np.ascontiguousarray(
                np.transpose(inputs[f"w_conv{i}"], (1, 2, 0))
            ).reshape(4, 128, 3, H)
            for i in (1, 2, 3, 4)
        ]
    ).astype(bf16)
    woutT = np.ascontiguousarray(inputs["w_out"].T).reshape(4, 128, 10).astype(bf16)
    bias_all = np.stack(
        [inputs["b_embed"], inputs["b_c1"], inputs["b_c2"]]
        + [inputs[f"b_conv{i}"] for i in range(5)]
    ).astype(f32)
    bias_pack = np.ascontiguousarray(
        bias_all.reshape(8, 4, 128).transpose(2, 0, 1)
    ).astype(f32)
    brep = np.ascontiguousarray(
        np.broadcast_to(inputs["b_out"].astype(f32), (128, NCHUNK, 10))
    )
    mask = np.zeros((128, 4), f32)
    mask[np.arange(128), np.arange(128) // 32] = 1.0

    in_maps = []
    for c in range(NCORES):
        xs = x[c * NB : (c + 1) * NB]
        z1 = xs[:, :T2]
        z2 = xs[:, T2:]
        z1T = np.ascontiguousarray(z1.reshape(NB * T2, 2).T).astype(bf16)
        z2r = np.ascontiguousarray(
            z2.reshape(NCHUNK, 4, T2, 2).transpose(1, 2, 0, 3)
        ).reshape(128, NCHUNK, 2)
        condT = np.ascontiguousarray(
            cond[c * NB : (c + 1) * NB].reshape(NB, 2 * COND).T
        ).astype(bf16)
        in_maps.append(
            dict(
                z1T=z1T,
                z2r=z2r,
                condT=condT,
                we=weT,
                wc1=wc1T,
                wc2=wc2T,
                wc0=wc0,
                wcv=wcv,
                wout=woutT,
                biases=bias_pack,
                brep=brep,
                mask=mask,
            )
        )
    return in_maps


def _assemble_output(x, results):
    z = np.empty((B, T, D), np.float32)
    ld = np.empty((B,), np.float32)
    for c in range(NCORES):
        z[c * NB : (c + 1) * NB, :T2] = x[c * NB : (c + 1) * NB, :T2]
        z2n = np.asarray(results[c]["z2n"], np.float32)
        z[c * NB : (c + 1) * NB, T2:] = (
            z2n.reshape(4, T2, NCHUNK, 2).transpose(2, 0, 1, 3).reshape(NB, T2, 2)
        )
        ld[c * NB : (c + 1) * NB] = np.asarray(results[c]["ld"], np.float32).reshape(
            NB
        )
    return z, ld


def run(inputs, trace=False, trace_cores=None):
    """Run on 8 NeuronCores; returns ((z, logdet), BassKernelResults)."""
    from concourse.bass_utils import run_bass_kernel_spmd

    nc = _get_program()
    in_maps = _host_inputs(inputs)
    res = run_bass_kernel_spmd(
        nc,
        in_maps,
        list(range(NCORES)),
        trace=trace,
        trace_cores=trace_cores if trace_cores is not None else list(range(NCORES)),
    )
    x = np.asarray(inputs["x"], np.float32)
    return _assemble_output(x, res.results), res


def kernel(**inputs):
    (z, ld), _ = run(inputs, trace=False)
    return z, ld


if __name__ == "__main__":
    print("build only:", _get_program())


# revision 8
# speedup vs baseline: 1.1785x; 1.1785x over previous
"""Trainium2 Bass kernel for nn_NlsqCond (ConvFlow NLSQ coupling layer).

Strategy: pure data parallel over batch B=256 -> 32 samples per core on 8
NeuronCores. Convs are computed as 3 shifted matmuls accumulating in PSUM
over a gap-column activation layout (34 columns per sample, zero guard
columns), so the k=3/pad=1 conv needs no boundary special-casing. Weights
are transposed host-side into lhsT layout and cast to bf16 (fp32 PSUM
accumulation); measured end-to-end error vs fp32 reference is ~1e-4.
The final projection is computed transposed ([cols, 10]) so the NLSQ
elementwise tail runs with full 128-partition parallelism; the per-sample
logdet partition-reduction is done with a small mask matmul.
"""

import math

import numpy as np

B, T, D, H, COND = 256, 64, 2, 512, 8
T2 = T // 2                      # 32
NCORES = 8
NB = B // NCORES                 # 32 samples per core
ST = T2 + 2                      # 34: per-sample column stride (zero gaps)
WCOLS = NB * ST                  # 1088
NG = 2                           # PSUM column groups per matmul set
GS = NB // NG                    # 16 samples per group
NCHUNK = NB * T2 // 128          # 8 column chunks of 128 for final proj
LOG_A = math.log(8.0 * math.sqrt(3.0) / 9.0 - 0.05)

_CACHE = {}


def _build_program():
    import concourse.bacc as bacc
    import concourse.mybir as mybir
    import concourse.tile as tile

    f32 = mybir.dt.float32
    bf16 = mybir.dt.bfloat16
    AF = mybir.ActivationFunctionType
    Alu = mybir.AluOpType

    nc = bacc.Bacc("TRN2", target_bir_lowering=False, debug=False)

    # ---- DRAM I/O ----
    d_z1T = nc.dram_tensor("z1T", [2, NB * T2], bf16, kind="ExternalInput")
    d_z2r = nc.dram_tensor("z2r", [128, NCHUNK, 2], f32, kind="ExternalInput")
    d_condT = nc.dram_tensor("condT", [2 * COND, NB], bf16, kind="ExternalInput")
    d_we = nc.dram_tensor("we", [2, H], bf16, kind="ExternalInput")
    d_wc1 = nc.dram_tensor("wc1", [2 * COND, H], bf16, kind="ExternalInput")
    d_wc2 = nc.dram_tensor("wc2", [4, 128, H], bf16, kind="ExternalInput")
    d_wc0 = nc.dram_tensor("wc0", [8, 128, 3, H], bf16, kind="ExternalInput")
    d_wcv = nc.dram_tensor("wcv", [4, 4, 128, 3, H], bf16, kind="ExternalInput")
    d_wout = nc.dram_tensor("wout", [4, 128, 10], bf16, kind="ExternalInput")
    d_bias = nc.dram_tensor("biases", [128, 8, 4], f32, kind="ExternalInput")
    d_brep = nc.dram_tensor("brep", [128, NCHUNK, 10], f32, kind="ExternalInput")
    d_mask = nc.dram_tensor("mask", [128, 4], f32, kind="ExternalInput")
    d_z2n = nc.dram_tensor("z2n", [128, NCHUNK, 2], f32, kind="ExternalOutput")
    d_ld = nc.dram_tensor("ld", [NCHUNK, 4], f32, kind="ExternalOutput")

    with tile.TileContext(nc) as tc:
        with (
            tc.tile_pool(name="w", bufs=1) as wp,
            tc.tile_pool(name="act", bufs=1) as ap_,
            tc.tile_pool(name="ew", bufs=1) as ewp,
            tc.tile_pool(name="ps", bufs=8, space="PSUM") as pp,
        ):
            def mk(pool, shape, dtype, tag):
                return pool.tile(shape, dtype, tag=tag, name=tag)

            # ---- SBUF loads (small inputs first so PE can start early) ----
            z1T_sb = mk(ap_, [2, NB * T2], bf16, "z1T")
            nc.sync.dma_start(z1T_sb[:], d_z1T.ap())
            condT_sb = mk(ap_, [2 * COND, NB], bf16, "condT")
            nc.sync.dma_start(condT_sb[:], d_condT.ap())
            we_sb = mk(wp, [2, H], bf16, "we")
            nc.sync.dma_start(we_sb[:], d_we.ap())
            wc1_sb = mk(wp, [2 * COND, H], bf16, "wc1")
            nc.sync.dma_start(wc1_sb[:], d_wc1.ap())
            bias_sb = mk(wp, [128, 8, 4], f32, "bias")
            nc.sync.dma_start(bias_sb[:], d_bias.ap())
            wc2_sb = [mk(wp, [128, H], bf16, f"wc2_{i}") for i in range(4)]
            for i in range(4):
                nc.sync.dma_start(wc2_sb[i][:], d_wc2.ap()[i])
            wc0_sb = [mk(wp, [128, 3, H], bf16, f"wc0_{i}") for i in range(8)]
            for i in range(8):
                nc.sync.dma_start(wc0_sb[i][:], d_wc0.ap()[i])
            wcv_sb = [
                [mk(wp, [128, 3, H], bf16, f"wcv_{l}_{i}") for i in range(4)]
                for l in range(4)
            ]
            for l in range(4):
                for i in range(4):
                    nc.sync.dma_start(wcv_sb[l][i][:], d_wcv.ap()[l, i])
            wout_sb = [mk(wp, [128, 10], bf16, f"wout_{i}") for i in range(4)]
            for i in range(4):
                nc.sync.dma_start(wout_sb[i][:], d_wout.ap()[i])
            brep_sb = mk(wp, [128, NCHUNK, 10], f32, "brep")
            nc.sync.dma_start(brep_sb[:], d_brep.ap())
            mask_sb = mk(wp, [128, 4], f32, "mask")
            nc.sync.dma_start(mask_sb[:], d_mask.ap())
            z2r_sb = mk(ewp, [128, NCHUNK, 2], f32, "z2r")
            nc.sync.dma_start(z2r_sb[:], d_z2r.ap())

            def bias_ap(idx, mt):
                return bias_sb[:, idx, mt : mt + 1]

            # ---- activation ping-pong buffers (gap layout) ----
            # Only the gap guard columns need zeroing; valid columns are
            # always written before they are read.
            actA = [mk(ap_, [128, WCOLS], bf16, f"A{i}") for i in range(8)]
            actB = [mk(ap_, [128, WCOLS], bf16, f"B{i}") for i in range(4)]
            actC = [mk(ap_, [128, WCOLS], bf16, f"C{i}") for i in range(4)]
            for ti, t_ in enumerate(actA + actB + actC):
                v = t_[:].rearrange("p (s c) -> p s c", c=ST)
                eng = nc.vector if ti % 2 else nc.gpsimd
                eng.memset(v[:, :, 0:1], 0.0)
                eng.memset(v[:, :, ST - 1 : ST], 0.0)

            def valid(tl, g=None):
                v = tl[:].rearrange("p (s c) -> p s c", c=ST)
                if g is None:
                    return v[:, :, 1 : 1 + T2]
                return v[:, g * GS : (g + 1) * GS, 1 : 1 + T2]

            # ---- embedding: h = z1 @ we.T + b_embed -> actA[0..3] ----
            for g in range(NG):
                for mt in range(4):
                    ps = mk(pp, [128, 512], f32, "ps")
                    nc.tensor.matmul(
                        ps[:],
                        we_sb[:, mt * 128 : (mt + 1) * 128],
                        z1T_sb[:, g * 512 : (g + 1) * 512],
                        start=True,
                        stop=True,
                    )
                    nc.scalar.activation(
                        valid(actA[mt], g),
                        ps[:].rearrange("p (s t) -> p s t", t=T2),
                        AF.Identity,
                        bias=bias_ap(0, mt),
                    )

            # ---- cond MLP: c2 = relu(W2 relu(W1 c + b1) + b2) ----
            c1_sb = [mk(ap_, [128, NB], bf16, f"c1_{i}") for i in range(4)]
            c2_sb = [mk(ap_, [128, NB], bf16, f"c2_{i}") for i in range(4)]
            for mt in range(4):
                ps = mk(pp, [128, 512], f32, "ps")
                nc.tensor.matmul(
                    ps[:, :NB],
                    wc1_sb[:, mt * 128 : (mt + 1) * 128],
                    condT_sb[:],
                    start=True,
                    stop=True,
                )
                nc.scalar.activation(
                    c1_sb[mt][:], ps[:, :NB], AF.Relu, bias=bias_ap(1, mt)
                )
            for mt in range(4):
                ps = mk(pp, [128, 512], f32, "ps")
                for kt in range(4):
                    nc.tensor.matmul(
                        ps[:, :NB],
                        wc2_sb[kt][:, mt * 128 : (mt + 1) * 128],
                        c1_sb[kt][:],
                        start=(kt == 0),
                        stop=(kt == 3),
                    )
                nc.scalar.activation(
                    c2_sb[mt][:], ps[:, :NB], AF.Relu, bias=bias_ap(2, mt)
                )
            # broadcast c2 over positions t -> actA[4..7]
            for mt in range(4):
                src = c2_sb[mt][:].unsqueeze(2).broadcast_to([128, NB, T2])
                nc.scalar.copy(valid(actA[4 + mt]), src)

            # ---- conv stack ----
            # Matmul moving operands must be single-free-dim, so each conv
            # matmul streams a contiguous window of the gap layout; outputs
            # at gap positions are garbage and simply never read back.
            WINDOWS = [(0, 15), (15, 15), (30, 2)]  # (sample base, n samples)
            srcs, ktn, wts = actA, 8, wc0_sb
            for L in range(5):
                dsts = actB if L % 2 == 0 else actC
                for mt in range(4):
                    pss = [mk(pp, [128, 512], f32, "ps") for _ in WINDOWS]
                    nacc = 3 * ktn
                    i = 0
                    for k in range(3):
                        for kt in range(ktn):
                            lhsT = wts[kt][:, k, mt * 128 : (mt + 1) * 128]
                            for wi, (sb, ns) in enumerate(WINDOWS):
                                n = ns * ST - 2
                                base = sb * ST + k
                                nc.tensor.matmul(
                                    pss[wi][:, :n],
                                    lhsT,
                                    srcs[kt][:, base : base + n],
                                    start=(i == 0),
                                    stop=(i == nacc - 1),
                                )
                            i += 1
                    for wi, (sb, ns) in enumerate(WINDOWS):
                        pv_ = pss[wi][:, : ns * ST].rearrange(
                            "p (s c) -> p s c", c=ST
                        )[:, :, 0:T2]
                        dv_ = dsts[mt][:].rearrange("p (s c) -> p s c", c=ST)[
                            :, sb : sb + ns, 1 : 1 + T2
                        ]
                        # split the PSUM->SBUF relu copies across ACT and DVE
                        if (mt * 3 + wi) % 2 == 0:
                            nc.scalar.activation(
                                dv_, pv_, AF.Relu, bias=bias_ap(3 + L, mt)
                            )
                        else:
                            nc.vector.tensor_scalar(
                                dv_,
                                pv_,
                                bias_ap(3 + L, mt),
                                0.0,
                                Alu.add,
                                Alu.max,
                            )
                srcs, ktn = dsts, 4
                if L < 4:
                    wts = wcv_sb[L]

            # ---- compact the final activation (drop gap columns) ----
            h5 = [mk(ap_, [128, NB * T2], bf16, f"h5_{i}") for i in range(4)]
            for kt in range(4):
                eng = nc.scalar if kt % 2 == 0 else nc.vector
                if kt % 2 == 0:
                    nc.scalar.copy(
                        h5[kt][:].rearrange("p (s t) -> p s t", t=T2),
                        valid(srcs[kt]),
                    )
                else:
                    nc.vector.tensor_copy(
                        h5[kt][:].rearrange("p (s t) -> p s t", t=T2),
                        valid(srcs[kt]),
                    )

            # ---- final projection (transposed): out[col, 10] ----
            pso = mk(pp, [128, 512], f32, "ps")
            for j in range(NCHUNK):
                for kt in range(4):
                    nc.tensor.matmul(
                        pso[:, j * 10 : (j + 1) * 10],
                        h5[kt][:, j * 128 : (j + 1) * 128],
                        wout_sb[kt][:],
                        start=(kt == 0),
                        stop=(kt == 3),
                    )

            # ---- NLSQ elementwise tail ----
            def ew(tag):
                return mk(ewp, [128, NCHUNK, 2], f32, tag)

            params = mk(ewp, [128, NCHUNK, 10], f32, "params")
            nc.vector.tensor_add(
                params[:],
                pso[:, : NCHUNK * 10].rearrange("p (j q) -> p j q", q=10),
                brep_sb[:],
            )
            pv = params[:].rearrange("p j (a q) -> p j a q", q=5)
            P0, P1, P2, P3, P4 = (pv[:, :, :, i] for i in range(5))

            loga_sb = mk(ewp, [128, 1], f32, "loga")
            nc.vector.memset(loga_sb[:], LOG_A)

            # group ACT functions (Exp x3, then Tanh, Ln last) to minimize
            # activation-table reloads
            u = ew("u")
            nc.vector.tensor_tensor(u[:], P1, P3, op=Alu.subtract)
            b_ = ew("b_")
            nc.scalar.activation(b_[:], P1, AF.Exp, scale=0.4)
            d_ = ew("d_")
            nc.scalar.activation(d_[:], P3, AF.Exp, scale=0.4)
            e = ew("e")
            nc.scalar.activation(e[:], u[:], AF.Exp, scale=0.4, bias=loga_sb[:])
            th = ew("th")
            nc.scalar.activation(th[:], P2, AF.Tanh, scale=0.3)
            c_ = ew("c_")
            nc.vector.tensor_mul(c_[:], th[:], e[:])
            t1 = ew("t1")
            nc.vector.tensor_mul(t1[:], d_[:], z2r_sb[:])
            arg = ew("arg")
            nc.vector.tensor_add(arg[:], t1[:], P4)
            sq = ew("sq")
            nc.vector.tensor_mul(sq[:], arg[:], arg[:])
            den = ew("den")
            nc.vector.tensor_scalar_add(den[:], sq[:], 1.0)
            rcp = ew("rcp")
            nc.vector.reciprocal(rcp[:], den[:])
            t2 = ew("t2")
            nc.vector.tensor_mul(t2[:], b_[:], z2r_sb[:])
            t3 = ew("t3")
            nc.vector.tensor_mul(t3[:], c_[:], rcp[:])
            s1 = ew("s1")
            nc.vector.tensor_add(s1[:], P0, t2[:])
            z2n_sb = ew("z2n_sb")
            nc.vector.tensor_add(z2n_sb[:], s1[:], t3[:])
            nc.sync.dma_start(d_z2n.ap(), z2n_sb[:])

            t4 = ew("t4")
            nc.vector.tensor_mul(t4[:], c_[:], d_[:])
            t5 = ew("t5")
            nc.vector.tensor_mul(t5[:], t4[:], arg[:])
            t6 = ew("t6")
            nc.vector.tensor_mul(t6[:], t5[:], rcp[:])
            t7 = ew("t7")
            nc.vector.tensor_mul(t7[:], t6[:], rcp[:])
            inner = ew("inner")
            nc.vector.scalar_tensor_tensor(
                inner[:], t7[:], -2.0, b_[:], op0=Alu.mult, op1=Alu.add
            )
            lg = ew("lg")
            nc.scalar.activation(lg[:], inner[:], AF.Ln)

            lg2 = mk(ewp, [128, NCHUNK], f32, "lg2")
            nc.vector.tensor_add(lg2[:], lg[:, :, 0], lg[:, :, 1])
            psl = mk(pp, [128, 512], f32, "ps")
            nc.tensor.matmul(
                psl[:NCHUNK, :4], lg2[:], mask_sb[:], start=True, stop=True
            )
            ld_sb = mk(ewp, [NCHUNK, 4], f32, "ld_sb")
            nc.vector.tensor_copy(ld_sb[:], psl[:NCHUNK, :4])
            nc.sync.dma_start(d_ld.ap(), ld_sb[:])

    nc.compile()
    return nc


def _get_program():
    if "nc" not in _CACHE:
        _CACHE["nc"] = _build_program()
    return _CACHE["nc"]


def _host_inputs(inputs):
    import ml_dtypes

    bf16 = ml_dtypes.bfloat16
    f32 = np.float32

    x = np.asarray(inputs["x"], f32)
    cond = np.asarray(inputs["cond"], f32)

    weT = np.ascontiguousarray(inputs["w_embed"][:, :2].T).astype(bf16)
    wc1T = np.ascontiguousarray(inputs["w_c1"].T).astype(bf16)
    wc2T = np.ascontiguousarray(inputs["w_c2"].T).reshape(4, 128, H).astype(bf16)
    wc0 = (
        np.ascontiguousarray(np.transpose(inputs["w_conv0"], (1, 2, 0)))
        .reshape(8, 128, 3, H)
        .astype(bf16)
    )
    wcv = np.stack(
        [
            np.ascontiguousarray(
                np.transpose(inputs[f"w_conv{i}"], (1, 2, 0))
            ).reshape(4, 128, 3, H)
            for i in (1, 2, 3, 4)
        ]
    ).astype(bf16)
    woutT = np.ascontiguousarray(inputs["w_out"].T).reshape(4, 128, 10).astype(bf16)
    bias_all = np.stack(
        [inputs["b_embed"], inputs["b_c1"], inputs["b_c2"]]
        + [inputs[f"b_conv{i}"] for i in range(5)]
    ).astype(f32)
    bias_pack = np.ascontiguousarray(
        bias_all.reshape(8, 4, 128).transpose(2, 0, 1)
    ).astype(f32)
    brep = np.ascontiguousarray(
        np.broadcast_to(inputs["b_out"].astype(f32), (128, NCHUNK, 10))
    )
    mask = np.zeros((128, 4), f32)
    mask[np.arange(128), np.arange(128) // 32] = 1.0

    in_maps = []
    for c in range(NCORES):
        xs = x[c * NB : (c + 1) * NB]
        z1 = xs[:, :T2]
        z2 = xs[:, T2:]
        z1T = np.ascontiguousarray(z1.reshape(NB * T2, 2).T).astype(bf16)
        z2r = np.ascontiguousarray(
            z2.reshape(NCHUNK, 4, T2, 2).transpose(1, 2, 0, 3)
        ).reshape(128, NCHUNK, 2)
        condT = np.ascontiguousarray(
            cond[c * NB : (c + 1) * NB].reshape(NB, 2 * COND).T
        ).astype(bf16)
        in_maps.append(
            dict(
                z1T=z1T,
                z2r=z2r,
                condT=condT,
                we=weT,
                wc1=wc1T,
                wc2=wc2T,
                wc0=wc0,
                wcv=wcv,
                wout=woutT,
                biases=bias_pack,
                brep=brep,
                mask=mask,
            )
        )
    return in_maps


def _assemble_output(x, results):
    z = np.empty((B, T, D), np.float32)
    ld = np.empty((B,), np.float32)
    for c in range(NCORES):
        z[c * NB : (c + 1) * NB, :T2] = x[c * NB : (c + 1) * NB, :T2]
        z2n = np.asarray(results[c]["z2n"], np.float32)
        z[c * NB : (c + 1) * NB, T2:] = (
            z2n.reshape(4, T2, NCHUNK, 2).transpose(2, 0, 1, 3).reshape(NB, T2, 2)
        )
        ld[c * NB : (c + 1) * NB] = np.asarray(results[c]["ld"], np.float32).reshape(
            NB
        )
    return z, ld


def run(inputs, trace=False, trace_cores=None):
    """Run on 8 NeuronCores; returns ((z, logdet), BassKernelResults)."""
    from concourse.bass_utils import run_bass_kernel_spmd

    nc = _get_program()
    in_maps = _host_inputs(inputs)
    res = run_bass_kernel_spmd(
        nc,
        in_maps,
        list(range(NCORES)),
        trace=trace,
        trace_cores=trace_cores if trace_cores is not None else list(range(NCORES)),
    )
    x = np.asarray(inputs["x"], np.float32)
    return _assemble_output(x, res.results), res


def kernel(**inputs):
    (z, ld), _ = run(inputs, trace=False)
    return z, ld


if __name__ == "__main__":
    print("build only:", _get_program())
